# revision 7
# baseline (speedup 1.0000x reference)
"""AdaptiveModulatedConv3d — 8-core TRN2 Bass kernel (v2).

Problem (hardcoded): BS=8, C_IN=C_OUT=64, K=3, STYLE_DIM=512, BANK=4,
D=H=W=32, pad=1, stride=1, f32 in/out.

Sharding: pure data-parallel over batch — each of the 8 NeuronCores gets one
sample, builds its per-sample demodulated conv weights on-device, and runs
its own 3D conv. No collectives.

Per-core conv: 3x3x3 conv as 27 shifted matmuls (contraction over C_IN=64)
accumulating into PSUM, with the PE array quadrant-packed 2x2 (row groups =
two d-plane halves of x, col groups = two output tiles per PSUM bank), so
four 64x64 matmuls stream concurrently.

v2 changes vs the previous kernel (156us):
 - split-plane x layout: partitions 0-63 hold d-planes 0..16, partitions
   64-127 hold planes 15..31 (no shifted second copy; casts run at full
   128-partition width; staging is per-plane and pipelined).
 - weight build off the critical path: demod computed from a host-side Gram
   matrix P[n,m,ci,co] via 16 tiny matmuls; bank mix fused to 4 vector ops
   per tap-chunk, alternating DVE/GpSimd; ACT tables preloaded; PE warmed
   with dummy matmuls so conv starts ~10us in, not ~37us.
 - drains strip the padded junk columns (strided PSUM read -> contiguous
   SBUF), so output DMAs are fully contiguous.
"""

import numpy as np

import concourse.bass as bass
import concourse.tile as tile
from concourse import bacc, mybir
from concourse import bass_utils

F32 = mybir.dt.float32
BF16 = mybir.dt.bfloat16

BS = 8
CI = 64
CO = 64
KK = 3
SD = 512
BANK = 4
D = H = W = 32
EPS = 1e-8
NCORES = 8

PLANE = (H + 2) * (W + 2)          # 1156: one zero-padded plane, flattened
LP = 17                            # d-planes stored per partition half
XLEN = 3 + LP * PLANE              # 2 front guard + planes + 1 back guard
ROWSPLIT = [(0, 11), (11, 11), (22, 10)]   # h-row tiles per d-plane
KCH = [(0, 11), (11, 19), (19, 27)]  # mix tap-chunks (koff ranges)

_CACHE = {}


def _conv_offsets(d):
    """Valid (kd, kh, kw) taps for output d-plane d, koff-ascending."""
    offs = []
    for kd in range(3):
        if 0 <= d + kd - 1 <= D - 1:
            for kh in range(3):
                for kw in range(3):
                    offs.append((kd, kh, kw))
    return offs


def _build():
    nc = bacc.Bacc("TRN2", target_bir_lowering=False, debug=False)
    x = nc.dram_tensor("x", [CI, D, H, W], F32, kind="ExternalInput").ap()
    wk = nc.dram_tensor("wk", [128, BANK], F32, kind="ExternalInput").ap()
    fw = nc.dram_tensor("fw", [128, BANK, BANK], F32,
                        kind="ExternalInput").ap()
    fb = nc.dram_tensor("fb", [1, BANK], F32, kind="ExternalInput").ap()
    mwt = nc.dram_tensor("mwt", [128, BANK, 128], F32,
                         kind="ExternalInput").ap()
    mb = nc.dram_tensor("mb", [128, 1], F32, kind="ExternalInput").ap()
    bankt = nc.dram_tensor("bankt", [CI, BANK, 27 * CO], BF16,
                           kind="ExternalInput").ap()
    gram = nc.dram_tensor("gram", [CI, BANK, BANK, CO], BF16,
                          kind="ExternalInput").ap()
    out = nc.dram_tensor("out", [CO, D, H, W], F32, kind="ExternalOutput").ap()

    with tile.TileContext(nc) as tc:
        with tc.tile_pool(name="singles", bufs=1) as sg, \
             tc.tile_pool(name="stg", bufs=6) as stg_pool, \
             tc.tile_pool(name="osb", bufs=4) as osb_pool:

            # ---- t~0: ACT table preloads + zero-fills (no DMA deps) ----
            warm = sg.tile([1, 2], F32)
            nc.vector.memset(warm, 0.0)
            nc.scalar.activation(warm[:, 0:1], warm[:, 0:1],
                                 mybir.ActivationFunctionType.Exp)
            nc.scalar.activation(warm[:, 1:2], warm[:, 1:2],
                                 mybir.ActivationFunctionType.Sqrt)
            ones1 = sg.tile([1, 128], F32)
            nc.vector.memset(ones1, 1.0)
            eps_sb = sg.tile([1, 1], F32)
            nc.vector.memset(eps_sb, EPS)

            xbf = sg.tile([128, XLEN], BF16)
            pl = xbf[:, 2:2 + LP * PLANE].rearrange(
                "p (d h w) -> p d h w", h=H + 2, w=W + 2)
            nc.vector.memset(xbf[:, 0:2], 0.0)
            nc.vector.memset(xbf[:, 2 + LP * PLANE:XLEN], 0.0)
            nc.vector.memset(pl[:, :, 0, :], 0.0)
            nc.vector.memset(pl[:, :, H + 1, :], 0.0)
            nc.vector.memset(pl[:, :, :, 0], 0.0)
            nc.vector.memset(pl[:, :, :, W + 1], 0.0)

            # ---- small param DMAs, gram, first bank chunk, first x ----
            wk_sb = sg.tile([128, BANK], F32)
            nc.sync.dma_start(out=wk_sb, in_=wk)
            fw_sb = sg.tile([128, BANK, BANK], F32)
            nc.sync.dma_start(out=fw_sb, in_=fw)
            fb_sb = sg.tile([1, BANK], F32)
            nc.sync.dma_start(out=fb_sb, in_=fb)
            mwt_sb = sg.tile([128, BANK, 128], F32)
            nc.sync.dma_start(out=mwt_sb, in_=mwt)
            mb_sb = sg.tile([128, 1], F32)
            nc.sync.dma_start(out=mb_sb, in_=mb)
            P_sb = sg.tile([CI, BANK, BANK, CO], BF16)
            nc.sync.dma_start(out=P_sb, in_=gram)

            bank_sb = sg.tile([128, BANK, 27 * CO], BF16)

            def bank_chunk_dma(ci_):
                k0, k1 = KCH[ci_]
                nc.sync.dma_start(out=bank_sb[0:64, :, k0 * CO:k1 * CO],
                                  in_=bankt[:, :, k0 * CO:k1 * CO])
                nc.sync.dma_start(out=bank_sb[64:128, :, k0 * CO:k1 * CO],
                                  in_=bankt[:, :, k0 * CO:k1 * CO])

            stg_tiles = {}

            def x_chunk_dma(c):
                stg = stg_pool.tile([128, H, W], F32)
                nc.sync.dma_start(out=stg[0:64], in_=x[:, c])
                nc.sync.dma_start(out=stg[64:128], in_=x[:, 15 + c])
                stg_tiles[c] = stg

            def x_chunk_cast(c, eng):
                b2 = 2 + c * PLANE + (W + 2) + 1
                dst = xbf[:, b2:b2 + H * (W + 2)].rearrange(
                    "p (h w) -> p h w", w=W + 2)[:, :, 0:W]
                if eng is nc.scalar:
                    nc.scalar.copy(dst, stg_tiles.pop(c))
                else:
                    eng.tensor_copy(dst, stg_tiles.pop(c))

            bank_chunk_dma(0)
            for c in range(3):
                x_chunk_dma(c)

            with tc.tile_pool(name="wpsum", bufs=2, space="PSUM") as wpsum:
                # ---- logits = w @ filter_w.T (+fb later); softmax without
                # normalization (uniform scale cancels through demod) ----
                ps_l = wpsum.tile([1, BANK], F32, tag="wps")
                for c in range(4):
                    nc.tensor.matmul(ps_l, lhsT=wk_sb[:, c:c + 1],
                                     rhs=fw_sb[:, c, :],
                                     start=(c == 0), stop=(c == 3))
                # ---- mod = w @ mod_w.T + mod_b, duplicated on both
                # partition halves (mwt free dim is ci twice) ----
                ps_m = wpsum.tile([128, 1], F32, tag="wps")
                for c in range(4):
                    nc.tensor.matmul(ps_m, lhsT=mwt_sb[:, c, :],
                                     rhs=wk_sb[:, c:c + 1],
                                     start=(c == 0), stop=(c == 3))

                logits = sg.tile([1, BANK], F32)
                nc.vector.tensor_add(logits, ps_l, fb_sb)
                fwt = sg.tile([1, BANK], F32)
                nc.scalar.activation(fwt, logits,
                                     mybir.ActivationFunctionType.Exp)
                mod_sb = sg.tile([128, 1], F32)
                nc.vector.tensor_add(mod_sb, ps_m, mb_sb)

                # fwt broadcast to all 128 partitions
                ps_fb = wpsum.tile([128, BANK], F32, tag="wps")
                nc.tensor.matmul(ps_fb, lhsT=ones1, rhs=fwt,
                                 start=True, stop=True)

                # per-partition mix coefficients: coef[p,n] = fwt_n * mod[p]
                coef = sg.tile([128, BANK], F32)
                nc.vector.tensor_scalar_mul(coef, ps_fb, mod_sb[:, 0:1])
                # demod coefficients (lower half only):
                # coefN[ci,n] = fwt_n * mod[ci]^2
                mod2 = sg.tile([64, 1], F32)
                nc.vector.tensor_mul(mod2, mod_sb[0:64], mod_sb[0:64])
                coefN = sg.tile([64, BANK], BF16)
                nc.vector.tensor_scalar_mul(coefN, ps_fb[0:64], mod2[:, 0:1])

                # ---- bank mix -> WT[128, 27, 64] bf16, in 3 tap chunks.
                # WT[p, koff, co] = sum_n coef[p,n] * bank[p, n, koff, co];
                # chunks 0,2 on DVE, chunk 1 on GpSimd. ----
                WT = sg.tile([128, 27, CO], BF16)
                mixtmp = [sg.tile([128, 11 * CO], F32, name=f"mxt{j}")
                          for j in range(3)]

                def mix_chunk(ci_, eng):
                    k0, k1 = KCH[ci_]
                    f0, f1 = k0 * CO, k1 * CO
                    nk = f1 - f0
                    tmp = mixtmp[ci_][:, 0:nk]
                    eng.tensor_scalar_mul(tmp, bank_sb[:, 0, f0:f1],
                                          coef[:, 0:1])
                    for n in range(1, 3):
                        eng.scalar_tensor_tensor(
                            out=tmp, in0=bank_sb[:, n, f0:f1],
                            scalar=coef[:, n:n + 1], in1=tmp,
                            op0=mybir.AluOpType.mult,
                            op1=mybir.AluOpType.add)
                    eng.scalar_tensor_tensor(
                        out=WT[:, k0:k1].rearrange("p k c -> p (k c)"),
                        in0=bank_sb[:, 3, f0:f1],
                        scalar=coef[:, 3:4], in1=tmp,
                        op0=mybir.AluOpType.mult, op1=mybir.AluOpType.add)

                mix_chunk(0, nc.vector)

                # first casts on ACT (idle until drains), rest on GpSimd
                for c in range(3):
                    x_chunk_cast(c, nc.scalar)
                bank_chunk_dma(1)
                for c in range(3, 5):
                    x_chunk_dma(c)
                mix_chunk(1, nc.vector)
                bank_chunk_dma(2)

                # ---- PE warmup (HAM unthrottle) on gram data ----
                Pflat = P_sb.rearrange("p n m c -> p (n m c)")
                warm_ps = wpsum.tile([128, 512], F32, tag="wps")
                for i in range(6):
                    nc.tensor.matmul(warm_ps[0:64], lhsT=Pflat[:, 0:64],
                                     rhs=Pflat[:, 0:512],
                                     start=True, stop=True)

                # ---- demod sum via Gram matrix:
                # dsum[co] = sum_m fwt_m * sum_n sum_ci coefN[ci,n] *
                #            P[ci,n,m,co] ----
                psD = wpsum.tile([1, BANK * CO], F32, tag="wps")
                for m in range(4):
                    for n in range(4):
                        nc.tensor.matmul(psD[0:1, m * CO:(m + 1) * CO],
                                         lhsT=coefN[:, n:n + 1],
                                         rhs=P_sb[:, n, m, :],
                                         start=(n == 0), stop=(n == 3))
                accA = sg.tile([1, CO], F32)
                accB = sg.tile([1, CO], F32)
                nc.vector.tensor_scalar_mul(accA, psD[0:1, 0:CO],
                                            fwt[:, 0:1])
                nc.vector.scalar_tensor_tensor(
                    out=accB, in0=psD[0:1, CO:2 * CO], scalar=fwt[:, 1:2],
                    in1=accA, op0=mybir.AluOpType.mult,
                    op1=mybir.AluOpType.add)
                nc.vector.scalar_tensor_tensor(
                    out=accA, in0=psD[0:1, 2 * CO:3 * CO], scalar=fwt[:, 2:3],
                    in1=accB, op0=mybir.AluOpType.mult,
                    op1=mybir.AluOpType.add)
                dsum = sg.tile([1, CO], F32)
                nc.vector.scalar_tensor_tensor(
                    out=dsum, in0=psD[0:1, 3 * CO:4 * CO], scalar=fwt[:, 3:4],
                    in1=accA, op0=mybir.AluOpType.mult,
                    op1=mybir.AluOpType.add)
                sstd = sg.tile([1, CO], F32)
                nc.scalar.activation(sstd, dsum,
                                     mybir.ActivationFunctionType.Sqrt,
                                     bias=eps_sb[:, 0:1])
                demod = sg.tile([1, CO], F32)
                nc.vector.reciprocal(demod, sstd)

                # mix chunk 2 emitted now (DVE queue, before demod DVE ops
                # consume psD)
                mix_chunk(2, nc.vector)

                # remaining staging: DMAs up-front (queues run ahead),
                # casts on GpSimd
                for c in range(5, LP):
                    x_chunk_dma(c)
                for c in range(3, LP):
                    x_chunk_cast(c, nc.gpsimd)

            # ---- conv ----
            dmT = sg.tile([CO, 1], F32)

            ltiles = [(0, d, r0, nr)
                      for d in list(range(1, 16)) + [0]
                      for (r0, nr) in ROWSPLIT]
            utiles = [(1, d, r0, nr)
                      for d in range(16, 32)
                      for (r0, nr) in ROWSPLIT]
            groups = [(ltiles[2 * i], utiles[2 * i],
                       ltiles[2 * i + 1], utiles[2 * i + 1])
                      for i in range(24)]
            # quadrant j: (rg, psum-tile idx, psum partition base)
            quads = [(0, 0, 0), (64, 0, 64), (0, 1, 0), (64, 1, 64)]

            with tc.tile_pool(name="cpsum", bufs=8, space="PSUM") as cp:
                for gi, group in enumerate(groups):
                    pss = [cp.tile([128, 512], F32, tag="cps",
                                   name=f"cps{gi % 4}_{j}") for j in range(2)]
                    osbG = osb_pool.tile([128, 2, 352], F32,
                                         name=f"osb{gi % 4}")
                    offs_l = [_conv_offsets(t[1]) for t in group]
                    nwaves = max(len(o) for o in offs_l)
                    for i in range(nwaves):
                        for j, (up, d, r0, nr) in enumerate(group):
                            offs = offs_l[j]
                            if i >= len(offs):
                                continue
                            kd, kh, kw = offs[i]
                            rg, pi, pb = quads[j]
                            koff = kd * 9 + kh * 3 + kw
                            slot = d + kd - 1 - (15 if up else 0)
                            off = 2 + slot * PLANE + (r0 + kh) * 34 \
                                + kw - 1
                            n = nr * 34
                            nc.tensor.matmul(
                                pss[pi][pb:pb + 64, 0:n],
                                lhsT=WT[rg:rg + 64, koff, :],
                                rhs=xbf[rg:rg + 64, off:off + n],
                                start=(i == 0), stop=(i == len(offs) - 1))
                    if gi == 0:
                        # demod transpose: dmT[co,1]; emitted after the
                        # first group so it doesn't gate conv start
                        ps_t = cp.tile([CO, 1], F32, tag="cps",
                                       name="ps_t")
                        nc.tensor.matmul(ps_t, lhsT=demod,
                                         rhs=ones1[:, 0:1],
                                         start=True, stop=True)
                        nc.vector.tensor_copy(dmT, ps_t)
                    for j, (up, d, r0, nr) in enumerate(group):
                        rg, pi, pb = quads[j]
                        n = nr * 34
                        slotj = j // 2
                        dst = osbG[pb:pb + 64, slotj, 0:nr * W].rearrange(
                            "p (r w) -> p r w", w=W)
                        src = pss[pi][pb:pb + 64, 0:n].rearrange(
                            "p (r w) -> p r w", w=34)[:, :, 1:W + 1]
                        nc.scalar.mul(dst, src, dmT[:, 0:1])
                        eng = nc.gpsimd if j % 2 else nc.sync
                        eng.dma_start(
                            out=out[:, d, r0:r0 + nr, :],
                            in_=osbG[pb:pb + 64, slotj,
                                     0:nr * W].rearrange(
                                "p (r w) -> p r w", w=W))

    nc.compile()
    return nc


def _shard_inputs(x, w, filter_w, filter_b, mod_w, mod_b, bank):
    """Host-side input marshalling: per-core shards + replicated params in
    the layouts the kernel expects."""
    import ml_dtypes
    fw_h = np.ascontiguousarray(
        filter_w.T.reshape(4, 128, BANK).transpose(1, 0, 2), np.float32)
    mwt_1 = mod_w.T.reshape(4, 128, CI).transpose(1, 0, 2)   # [128, 4, 64]
    mwt_h = np.ascontiguousarray(
        np.concatenate([mwt_1, mwt_1], axis=2), np.float32)  # [128, 4, 128]
    bank_h = np.ascontiguousarray(
        bank.reshape(BANK, CO, CI, 27).transpose(2, 0, 3, 1)
        .reshape(CI, BANK, 27 * CO)).astype(ml_dtypes.bfloat16)
    fb_h = np.ascontiguousarray(filter_b.reshape(1, BANK), np.float32)
    mb_h = np.ascontiguousarray(
        np.concatenate([mod_b, mod_b]).reshape(128, 1), np.float32)
    # Gram matrix for the demodulation sum:
    # P[ci, n, m, co] = sum_k bank[n, co, ci, k] * bank[m, co, ci, k]
    bk = np.asarray(bank, np.float32).reshape(BANK, CO, CI, 27)
    gram_h = np.einsum('nuik,muik->inmu', bk, bk)
    gram_h = np.ascontiguousarray(gram_h).astype(ml_dtypes.bfloat16)
    in_maps = []
    for i in range(NCORES):
        in_maps.append({
            "x": np.ascontiguousarray(x[i], np.float32),
            "wk": np.ascontiguousarray(w[i].reshape(4, 128).T, np.float32),
            "fw": fw_h, "fb": fb_h, "mwt": mwt_h, "mb": mb_h,
            "bankt": bank_h, "gram": gram_h,
        })
    return in_maps


def _run(inputs, trace=False):
    if "nc" not in _CACHE:
        _CACHE["nc"] = _build()
    nc = _CACHE["nc"]
    in_maps = _shard_inputs(**inputs)
    res = bass_utils.run_bass_kernel_spmd(
        nc, in_maps, core_ids=list(range(NCORES)), trace=trace)
    out = np.stack([res.results[i]["out"] for i in range(NCORES)])
    return out.astype(np.float32), res


def kernel(**inputs):
    out, _ = _run(inputs, trace=False)
    return out


# revision 8
# speedup vs baseline: 1.5985x; 1.5985x over previous
"""AdaptiveModulatedConv3d — 8-core TRN2 Bass kernel (v2).

Problem (hardcoded): BS=8, C_IN=C_OUT=64, K=3, STYLE_DIM=512, BANK=4,
D=H=W=32, pad=1, stride=1, f32 in/out.

Sharding: pure data-parallel over batch — each of the 8 NeuronCores gets one
sample, builds its per-sample demodulated conv weights on-device, and runs
its own 3D conv. No collectives.

Per-core conv: 3x3x3 conv as 27 shifted matmuls (contraction over C_IN=64)
accumulating into PSUM, with the PE array quadrant-packed 2x2 (row groups =
two d-plane halves of x, col groups = two output tiles per PSUM bank), so
four 64x64 matmuls stream concurrently.

v2 changes vs the previous kernel (156us):
 - split-plane x layout: partitions 0-63 hold d-planes 0..16, partitions
   64-127 hold planes 15..31 (no shifted second copy; casts run at full
   128-partition width; staging is per-plane and pipelined).
 - weight build off the critical path: demod computed from a host-side Gram
   matrix P[n,m,ci,co] via 16 tiny matmuls; bank mix fused to 4 vector ops
   per tap-chunk, alternating DVE/GpSimd; ACT tables preloaded; PE warmed
   with dummy matmuls so conv starts ~10us in, not ~37us.
 - drains strip the padded junk columns (strided PSUM read -> contiguous
   SBUF), so output DMAs are fully contiguous.
"""

import numpy as np

import concourse.bass as bass
import concourse.tile as tile
from concourse import bacc, mybir
from concourse import bass_utils

F32 = mybir.dt.float32
BF16 = mybir.dt.bfloat16

BS = 8
CI = 64
CO = 64
KK = 3
SD = 512
BANK = 4
D = H = W = 32
EPS = 1e-8
NCORES = 8

PLANE = (H + 2) * (W + 2)          # 1156: one zero-padded plane, flattened
LP = 17                            # d-planes stored per partition half
XLEN = 3 + LP * PLANE              # 2 front guard + planes + 1 back guard
ROWSPLIT = [(0, 11), (11, 11), (22, 10)]   # h-row tiles per d-plane
KCH = [(0, 11), (11, 19), (19, 27)]  # mix tap-chunks (koff ranges)

_CACHE = {}


def _conv_offsets(d):
    """Valid (kd, kh, kw) taps for output d-plane d, koff-ascending."""
    offs = []
    for kd in range(3):
        if 0 <= d + kd - 1 <= D - 1:
            for kh in range(3):
                for kw in range(3):
                    offs.append((kd, kh, kw))
    return offs


def _build():
    nc = bacc.Bacc("TRN2", target_bir_lowering=False, debug=False)
    x = nc.dram_tensor("x", [CI, D, H, W], F32, kind="ExternalInput").ap()
    wk = nc.dram_tensor("wk", [128, BANK], F32, kind="ExternalInput").ap()
    fw = nc.dram_tensor("fw", [128, BANK, BANK], F32,
                        kind="ExternalInput").ap()
    fb = nc.dram_tensor("fb", [1, BANK], F32, kind="ExternalInput").ap()
    mwt = nc.dram_tensor("mwt", [128, BANK, 128], F32,
                         kind="ExternalInput").ap()
    mb = nc.dram_tensor("mb", [128, 1], F32, kind="ExternalInput").ap()
    bankt = nc.dram_tensor("bankt", [CI, BANK, 27 * CO], BF16,
                           kind="ExternalInput").ap()
    gram = nc.dram_tensor("gram", [CI, BANK, BANK, CO], BF16,
                          kind="ExternalInput").ap()
    out = nc.dram_tensor("out", [CO, D, H, W], F32, kind="ExternalOutput").ap()

    with tile.TileContext(nc) as tc:
        with tc.tile_pool(name="singles", bufs=1) as sg, \
             tc.tile_pool(name="stg", bufs=6) as stg_pool, \
             tc.tile_pool(name="osb", bufs=4) as osb_pool:

            # ---- t~0: ACT table preloads + zero-fills (no DMA deps) ----
            warm = sg.tile([1, 2], F32)
            nc.vector.memset(warm, 0.0)
            nc.scalar.activation(warm[:, 0:1], warm[:, 0:1],
                                 mybir.ActivationFunctionType.Exp)
            ones1 = sg.tile([1, 128], F32)
            nc.vector.memset(ones1, 1.0)
            eps_sb = sg.tile([1, 1], F32)
            nc.vector.memset(eps_sb, EPS)

            xbf = sg.tile([128, XLEN], BF16)
            pl = xbf[:, 2:2 + LP * PLANE].rearrange(
                "p (d h w) -> p d h w", h=H + 2, w=W + 2)
            nc.gpsimd.memset(xbf[:, 0:2], 0.0)
            nc.gpsimd.memset(xbf[:, 2 + LP * PLANE:XLEN], 0.0)
            nc.gpsimd.memset(pl[:, :, 0, :], 0.0)
            nc.gpsimd.memset(pl[:, :, H + 1, :], 0.0)
            nc.gpsimd.memset(pl[:, :, :, 0], 0.0)
            nc.gpsimd.memset(pl[:, :, :, W + 1], 0.0)

            # ---- small param DMAs, gram, first bank chunk, first x ----
            wk_sb = sg.tile([128, BANK], F32)
            nc.sync.dma_start(out=wk_sb, in_=wk)
            fw_sb = sg.tile([128, BANK, BANK], F32)
            nc.sync.dma_start(out=fw_sb, in_=fw)
            fb_sb = sg.tile([1, BANK], F32)
            nc.sync.dma_start(out=fb_sb, in_=fb)
            mwt_sb = sg.tile([128, BANK, 128], F32)
            nc.sync.dma_start(out=mwt_sb, in_=mwt)
            mb_sb = sg.tile([128, 1], F32)
            nc.sync.dma_start(out=mb_sb, in_=mb)
            P_sb = sg.tile([CI, BANK, BANK, CO], BF16)
            nc.sync.dma_start(out=P_sb, in_=gram)

            bank_sb = sg.tile([128, BANK, 27 * CO], BF16)

            def bank_chunk_dma(ci_):
                k0, k1 = KCH[ci_]
                nc.sync.dma_start(out=bank_sb[0:64, :, k0 * CO:k1 * CO],
                                  in_=bankt[:, :, k0 * CO:k1 * CO])
                nc.sync.dma_start(out=bank_sb[64:128, :, k0 * CO:k1 * CO],
                                  in_=bankt[:, :, k0 * CO:k1 * CO])

            stg_tiles = {}

            def x_chunk_dma(c):
                stg = stg_pool.tile([128, H, W], F32)
                nc.sync.dma_start(out=stg[0:64], in_=x[:, c])
                nc.sync.dma_start(out=stg[64:128], in_=x[:, 15 + c])
                stg_tiles[c] = stg

            def x_chunk_cast(c, eng):
                b2 = 2 + c * PLANE + (W + 2) + 1
                dst = xbf[:, b2:b2 + H * (W + 2)].rearrange(
                    "p (h w) -> p h w", w=W + 2)[:, :, 0:W]
                if eng is nc.scalar:
                    nc.scalar.copy(dst, stg_tiles.pop(c))
                else:
                    eng.tensor_copy(dst, stg_tiles.pop(c))

            bank_chunk_dma(0)
            for c in range(3):
                x_chunk_dma(c)

            with tc.tile_pool(name="wpsum", bufs=2, space="PSUM") as wpsum:
                # ---- logits = w @ filter_w.T (+fb later); softmax without
                # normalization (uniform scale cancels through demod) ----
                ps_l = wpsum.tile([1, BANK], F32, tag="wps")
                for c in range(4):
                    nc.tensor.matmul(ps_l, lhsT=wk_sb[:, c:c + 1],
                                     rhs=fw_sb[:, c, :],
                                     start=(c == 0), stop=(c == 3))
                # ---- mod = w @ mod_w.T + mod_b, duplicated on both
                # partition halves (mwt free dim is ci twice) ----
                ps_m = wpsum.tile([128, 1], F32, tag="wps")
                for c in range(4):
                    nc.tensor.matmul(ps_m, lhsT=mwt_sb[:, c, :],
                                     rhs=wk_sb[:, c:c + 1],
                                     start=(c == 0), stop=(c == 3))

                logits = sg.tile([1, BANK], F32)
                nc.vector.tensor_add(logits, ps_l, fb_sb)
                fwt = sg.tile([1, BANK], F32)
                nc.scalar.activation(fwt, logits,
                                     mybir.ActivationFunctionType.Exp)
                mod_sb = sg.tile([128, 1], F32)
                nc.vector.tensor_add(mod_sb, ps_m, mb_sb)

                # fwt broadcast to all 128 partitions
                ps_fb = wpsum.tile([128, BANK], F32, tag="wps")
                nc.tensor.matmul(ps_fb, lhsT=ones1, rhs=fwt,
                                 start=True, stop=True)

                # per-partition mix coefficients: coef[p,n] = fwt_n * mod[p]
                coef = sg.tile([128, BANK], F32)
                nc.vector.tensor_scalar_mul(coef, ps_fb, mod_sb[:, 0:1])
                # demod coefficients (lower half only):
                # coefN[ci,n] = fwt_n * mod[ci]^2
                mod2 = sg.tile([64, 1], F32)
                nc.vector.tensor_mul(mod2, mod_sb[0:64], mod_sb[0:64])
                coefN = sg.tile([64, BANK], BF16)
                nc.vector.tensor_scalar_mul(coefN, ps_fb[0:64], mod2[:, 0:1])

                # ---- bank mix -> WT[128, 27, 64] bf16, in 3 tap chunks.
                # WT[p, koff, co] = sum_n coef[p,n] * bank[p, n, koff, co];
                # chunks 0,2 on DVE, chunk 1 on GpSimd. ----
                WT = sg.tile([128, 27, CO], BF16)
                mixtmp = [sg.tile([128, 11 * CO], BF16, name=f"mxt{j}")
                          for j in range(3)]

                def mix_chunk(ci_, eng):
                    k0, k1 = KCH[ci_]
                    f0, f1 = k0 * CO, k1 * CO
                    nk = f1 - f0
                    tmp = mixtmp[ci_][:, 0:nk]
                    eng.tensor_scalar_mul(tmp, bank_sb[:, 0, f0:f1],
                                          coef[:, 0:1])
                    for n in range(1, 3):
                        eng.scalar_tensor_tensor(
                            out=tmp, in0=bank_sb[:, n, f0:f1],
                            scalar=coef[:, n:n + 1], in1=tmp,
                            op0=mybir.AluOpType.mult,
                            op1=mybir.AluOpType.add)
                    eng.scalar_tensor_tensor(
                        out=WT[:, k0:k1].rearrange("p k c -> p (k c)"),
                        in0=bank_sb[:, 3, f0:f1],
                        scalar=coef[:, 3:4], in1=tmp,
                        op0=mybir.AluOpType.mult, op1=mybir.AluOpType.add)

                # first casts on DVE (fast), before mix chunk 0;
                # the rest go to GpSimd
                for c in range(3):
                    x_chunk_cast(c, nc.vector)
                mix_chunk(0, nc.vector)
                bank_chunk_dma(1)
                for c in range(3, 5):
                    x_chunk_dma(c)
                mix_chunk(1, nc.vector)
                bank_chunk_dma(2)

                # ---- PE warmup (HAM unthrottle) on gram data ----
                Pflat = P_sb.rearrange("p n m c -> p (n m c)")
                warm_ps = wpsum.tile([128, 512], F32, tag="wps")
                for i in range(6):
                    nc.tensor.matmul(warm_ps[0:64], lhsT=Pflat[:, 0:64],
                                     rhs=Pflat[:, 0:512],
                                     start=True, stop=True)

                # ---- demod sum via Gram matrix:
                # dsum[co] = sum_m fwt_m * sum_n sum_ci coefN[ci,n] *
                #            P[ci,n,m,co] ----
                psD = wpsum.tile([1, BANK * CO], F32, tag="wps")
                for m in range(4):
                    for n in range(4):
                        nc.tensor.matmul(psD[0:1, m * CO:(m + 1) * CO],
                                         lhsT=coefN[:, n:n + 1],
                                         rhs=P_sb[:, n, m, :],
                                         start=(n == 0), stop=(n == 3))
                accA = sg.tile([1, CO], F32)
                accB = sg.tile([1, CO], F32)
                nc.vector.tensor_scalar_mul(accA, psD[0:1, 0:CO],
                                            fwt[:, 0:1])
                nc.vector.scalar_tensor_tensor(
                    out=accB, in0=psD[0:1, CO:2 * CO], scalar=fwt[:, 1:2],
                    in1=accA, op0=mybir.AluOpType.mult,
                    op1=mybir.AluOpType.add)
                nc.vector.scalar_tensor_tensor(
                    out=accA, in0=psD[0:1, 2 * CO:3 * CO], scalar=fwt[:, 2:3],
                    in1=accB, op0=mybir.AluOpType.mult,
                    op1=mybir.AluOpType.add)
                dsum = sg.tile([1, CO], F32)
                nc.vector.scalar_tensor_tensor(
                    out=dsum, in0=psD[0:1, 3 * CO:4 * CO], scalar=fwt[:, 3:4],
                    in1=accA, op0=mybir.AluOpType.mult,
                    op1=mybir.AluOpType.add)
                sstd = sg.tile([1, CO], F32)
                nc.scalar.activation(sstd, dsum,
                                     mybir.ActivationFunctionType.Sqrt,
                                     bias=eps_sb[:, 0:1])
                demod = sg.tile([1, CO], F32)
                nc.vector.reciprocal(demod, sstd)

                # mix chunk 2 emitted now (DVE queue, before demod DVE ops
                # consume psD)
                mix_chunk(2, nc.vector)

                # remaining staging: DMAs up-front (queues run ahead),
                # casts on GpSimd
                for c in range(5, LP):
                    x_chunk_dma(c)
                for c in range(3, LP):
                    x_chunk_cast(c, nc.gpsimd)

            # ---- conv ----
            dmT = sg.tile([CO, 1], F32)

            ltiles = [(0, d, r0, nr)
                      for d in list(range(1, 16)) + [0]
                      for (r0, nr) in ROWSPLIT]
            utiles = [(1, d, r0, nr)
                      for d in range(16, 32)
                      for (r0, nr) in ROWSPLIT]
            groups = [(ltiles[2 * i], utiles[2 * i],
                       ltiles[2 * i + 1], utiles[2 * i + 1])
                      for i in range(24)]
            # quadrant j: (rg, psum-tile idx, psum partition base).
            # All four PE quadrants (rg, pb) distinct so the four matmuls
            # stream concurrently; two psum banks per group (L-tiles in one,
            # U-tiles in the other).
            quads = [(0, 0, 0), (64, 1, 0), (0, 0, 64), (64, 1, 64)]

            with tc.tile_pool(name="cpsum", bufs=8, space="PSUM") as cp:
                for gi, group in enumerate(groups):
                    pss = [cp.tile([128, 512], F32, tag="cps",
                                   name=f"cps{gi % 4}_{j}") for j in range(2)]
                    osbG = osb_pool.tile([128, 2, 352], F32,
                                         name=f"osb{gi % 4}")
                    offs_l = [_conv_offsets(t[1]) for t in group]
                    nwaves = max(len(o) for o in offs_l)
                    for i in range(nwaves):
                        for j, (up, d, r0, nr) in enumerate(group):
                            offs = offs_l[j]
                            if i >= len(offs):
                                continue
                            kd, kh, kw = offs[i]
                            rg, pi, pb = quads[j]
                            koff = kd * 9 + kh * 3 + kw
                            slot = d + kd - 1 - (15 if up else 0)
                            off = 2 + slot * PLANE + (r0 + kh) * 34 \
                                + kw - 1
                            n = nr * 34
                            nc.tensor.matmul(
                                pss[pi][pb:pb + 64, 0:n],
                                lhsT=WT[rg:rg + 64, koff, :],
                                rhs=xbf[rg:rg + 64, off:off + n],
                                start=(i == 0), stop=(i == len(offs) - 1))
                    if gi == 0:
                        # demod transpose: dmT[co,1]; emitted after the
                        # first group so it doesn't gate conv start
                        ps_t = cp.tile([CO, 1], F32, tag="cps",
                                       name="ps_t")
                        nc.tensor.matmul(ps_t, lhsT=demod,
                                         rhs=ones1[:, 0:1],
                                         start=True, stop=True)
                        nc.vector.tensor_copy(dmT, ps_t)
                    for j, (up, d, r0, nr) in enumerate(group):
                        rg, pi, pb = quads[j]
                        n = nr * 34
                        slotj = j % 2
                        dst = osbG[pb:pb + 64, slotj, 0:nr * W].rearrange(
                            "p (r w) -> p r w", w=W)
                        src = pss[pi][pb:pb + 64, 0:n].rearrange(
                            "p (r w) -> p r w", w=34)[:, :, 1:W + 1]
                        nc.scalar.mul(dst, src, dmT[:, 0:1])
                        eng = nc.gpsimd if j % 2 else nc.sync
                        eng.dma_start(
                            out=out[:, d, r0:r0 + nr, :],
                            in_=osbG[pb:pb + 64, slotj,
                                     0:nr * W].rearrange(
                                "p (r w) -> p r w", w=W))

    nc.compile()
    return nc


def _shard_inputs(x, w, filter_w, filter_b, mod_w, mod_b, bank):
    """Host-side input marshalling: per-core shards + replicated params in
    the layouts the kernel expects."""
    import ml_dtypes
    fw_h = np.ascontiguousarray(
        filter_w.T.reshape(4, 128, BANK).transpose(1, 0, 2), np.float32)
    mwt_1 = mod_w.T.reshape(4, 128, CI).transpose(1, 0, 2)   # [128, 4, 64]
    mwt_h = np.ascontiguousarray(
        np.concatenate([mwt_1, mwt_1], axis=2), np.float32)  # [128, 4, 128]
    bank_h = np.ascontiguousarray(
        bank.reshape(BANK, CO, CI, 27).transpose(2, 0, 3, 1)
        .reshape(CI, BANK, 27 * CO)).astype(ml_dtypes.bfloat16)
    fb_h = np.ascontiguousarray(filter_b.reshape(1, BANK), np.float32)
    mb_h = np.ascontiguousarray(
        np.concatenate([mod_b, mod_b]).reshape(128, 1), np.float32)
    # Gram matrix for the demodulation sum:
    # P[ci, n, m, co] = sum_k bank[n, co, ci, k] * bank[m, co, ci, k]
    bk = np.asarray(bank, np.float32).reshape(BANK, CO, CI, 27)
    gram_h = np.einsum('nuik,muik->inmu', bk, bk)
    gram_h = np.ascontiguousarray(gram_h).astype(ml_dtypes.bfloat16)
    in_maps = []
    for i in range(NCORES):
        in_maps.append({
            "x": np.ascontiguousarray(x[i], np.float32),
            "wk": np.ascontiguousarray(w[i].reshape(4, 128).T, np.float32),
            "fw": fw_h, "fb": fb_h, "mwt": mwt_h, "mb": mb_h,
            "bankt": bank_h, "gram": gram_h,
        })
    return in_maps


def _run(inputs, trace=False):
    if "nc" not in _CACHE:
        _CACHE["nc"] = _build()
    nc = _CACHE["nc"]
    in_maps = _shard_inputs(**inputs)
    res = bass_utils.run_bass_kernel_spmd(
        nc, in_maps, core_ids=list(range(NCORES)), trace=trace)
    out = np.stack([res.results[i]["out"] for i in range(NCORES)])
    return out.astype(np.float32), res


def kernel(**inputs):
    out, _ = _run(inputs, trace=False)
    return out
